# revision 32
# baseline (speedup 1.0000x reference)
"""Trainium2 Bass kernel: Wan-style interleaved RoPE on q/k + causal attention.

Full problem: q,k,v [B=2, S=2048, H=16, D=128] fp32, freqs [1, S, 1, D].
  rq = rope(q), rk = rope(k)
  out[b,h,q,d] = softmax_causal(rq @ rk^T / sqrt(D)) @ v      -> [B, H, S, D]

Sharding: heads split across 8 cores (2 heads/core); each core handles
4 independent (b, h) attention problems. Inputs are sliced on host, the
SPMD kernel runs on cores 0-7, outputs are concatenated on host.

Layout trick: scores = sum_d rq[d]*rk[d] are invariant under any shared
permutation of d, so q and k are shipped de-interleaved (evens then
odds) AND pre-transposed to [D', S] on the host.  The vector engines
have no cross-partition path, so the even/odd halves are loaded
DUPLICATED (two half-DMAs from the same DRAM rows):
  qE = (x0|x0), qO = (x1|x1), FF = (f0|f1), GG = (-f1|f0)
  rqT' = qE*FF + qO*GG
       = (f0*x0 - f1*x1 | f1*x0 + f0*x1)   -- exactly interleaved RoPE
       in the shared (evens|odds) d-order.
No PE transposes of q/k are needed; v keeps the standard d-order so
the PV output comes out in natural [q, d] order.

Attention per (b,h), per q-block of 512 columns:
  scoresT[k, q] = rkT'_tile.T @ rqT'_block   (transposed scores, N<=512,
    partial-N on diagonal tiles -- only the causal area is computed)
  causal tri-mask added in-PSUM by a small bf16 identity@mask matmul
  softmax without max-subtraction (scores bounded, exp finite in fp32)
  exp on ACT batched over 2 psum banks -> probT (float32r)
  outT[d, q] += v_tile.T @ probT   (v stationary, N<=512)
  sums[1, q] += ones.T @ probT
  PE-transpose outT/sums back, multiply by reciprocal sums, DMA out.

Matmuls run in float32r (reduced-precision fp32, ~2x faster than fp32).
"""

import math

import numpy as np

B, S, H, D = 2, 2048, 16, 128
NCORES = 8
HPC = H // NCORES          # heads per core
NBH = B * HPC              # (b, h) problems per core
NT = S // 128              # s-tiles
QB = S // 512              # q blocks of 512
SCALE = 1.0 / math.sqrt(D)
NEG = -1e30
SB = 2                     # scores batch (tiles of 512 q-cols per exp batch)
USE_F32R = True            # float32r matmuls (2x faster PE, ~6e-4 rel err)

_CACHE = {}


def _build():
    import concourse.mybir as mybir
    import concourse.tile as tile
    from concourse import bacc
    from concourse.masks import make_identity

    f32 = mybir.dt.float32
    f32r = mybir.dt.float32r
    bf16 = mybir.dt.bfloat16
    Alu = mybir.AluOpType
    Act = mybir.ActivationFunctionType

    nc = bacc.Bacc("TRN2", target_bir_lowering=False, debug=False,
                   num_devices=NCORES)
    qd = nc.dram_tensor("qT", [NBH, D, S], f32, kind="ExternalInput")
    kd = nc.dram_tensor("kT", [NBH, D, S], f32, kind="ExternalInput")
    vd = nc.dram_tensor("v", [NBH, S, D], f32, kind="ExternalInput")
    fd = nc.dram_tensor("freqsT", [D, S], f32, kind="ExternalInput")
    gd = nc.dram_tensor("freqsG", [D, S], f32, kind="ExternalInput")
    od = nc.dram_tensor("out", [NBH, S, D], f32, kind="ExternalOutput")

    mdt = f32r if USE_F32R else f32

    with tile.TileContext(nc) as tc:
        with (
            tc.tile_pool(name="const", bufs=1) as cpool,
            tc.tile_pool(name="io", bufs=2) as iopool,
            tc.tile_pool(name="rope", bufs=4) as rpool,
            tc.tile_pool(name="xt", bufs=2) as xtpool,
            tc.tile_pool(name="prob", bufs=3) as ppool,
            tc.tile_pool(name="small", bufs=2) as spool,
            tc.tile_pool(name="sc_ps", bufs=2, space="PSUM") as sc_ps,
            tc.tile_pool(name="out_ps", bufs=2, space="PSUM") as out_ps,
            tc.tile_pool(name="mp_ps", bufs=2, space="PSUM") as mp_ps,
        ):
            # ---- constants ----
            ident = cpool.tile([128, 128], f32, tag="ident")
            make_identity(nc, ident[:])
            ident_bf = cpool.tile([128, 128], bf16, tag="ident_bf")
            nc.vector.tensor_copy(ident_bf[:], ident[:])
            # tri_bf[k, t] = 0 where k <= t (valid), NEG where k > t.
            tri_bf = cpool.tile([128, 128], bf16, tag="tri_bf")
            nc.gpsimd.memset(tri_bf[:], 0.0)
            nc.gpsimd.affine_select(
                out=tri_bf[:], in_=tri_bf[:],
                compare_op=Alu.is_ge, fill=NEG, base=0,
                pattern=[[1, 128]], channel_multiplier=-1,
            )
            ones_f32 = cpool.tile([128, 1], f32, tag="ones_f32")
            nc.gpsimd.memset(ones_f32[:], 1.0)
            ones_col = cpool.tile([128, 1], mdt, tag="ones")
            nc.vector.tensor_copy(ones_col[:], ones_f32[:])
            # FF = (f0|f1), GG = (-f1|f0) -- host precomputed.
            FF = cpool.tile([128, S], f32, tag="FF")
            GG = cpool.tile([128, S], f32, tag="GG")
            nc.sync.dma_start(FF[:], fd.ap())
            nc.sync.dma_start(GG[:], gd.ap())

            for bh in range(NBH):
                # ---- load ----
                v_sb = iopool.tile([128, S], f32, tag="v")
                nc.sync.dma_start(
                    v_sb[:].rearrange("p (t d) -> p t d", d=D),
                    vd.ap()[bh].rearrange("(t p) d -> p t d", p=128),
                )
                v_mm = iopool.tile([128, S], mdt, tag="v_mm")
                nc.vector.tensor_copy(v_mm[:], v_sb[:])

                # ---- RoPE in transposed layout, chunked so the first
                # q-block's matmuls can start before the whole (b,h) is
                # roped (separate tiles per 512-col chunk => fine-grained
                # dependencies) ----
                qTc = [xtpool.tile([128, 512], mdt, tag=f"qT{c}", name=f"qT{c}")
                       for c in range(4)]
                kTc = [xtpool.tile([128, 512], mdt, tag=f"kT{c}", name=f"kT{c}")
                       for c in range(4)]
                for c in range(4):
                    cs = slice(c * 512, (c + 1) * 512)
                    for xd, xTl, teven, todd in (
                            (kd, kTc, "kE", "kO"), (qd, qTc, "qE", "qO")):
                        xE = rpool.tile([128, 512], f32, tag=teven)
                        xO = rpool.tile([128, 512], f32, tag=todd)
                        nc.sync.dma_start(xE[0:64, :], xd.ap()[bh, 0:64, cs])
                        nc.sync.dma_start(xE[64:128, :],
                                          xd.ap()[bh, 0:64, cs])
                        nc.sync.dma_start(xO[0:64, :],
                                          xd.ap()[bh, 64:128, cs])
                        nc.sync.dma_start(xO[64:128, :],
                                          xd.ap()[bh, 64:128, cs])
                        nc.vector.tensor_mul(xE[:], xE[:], FF[:, cs])
                        nc.gpsimd.tensor_mul(xO[:], xO[:], GG[:, cs])
                        nc.vector.tensor_add(xTl[c][:], xE[:], xO[:])

                # ---- attention per q-block ----
                for qb in range(QB):
                    nk = 4 * qb + 4
                    outT = out_ps.tile([128, 512], f32, tag="outT")
                    sums = mp_ps.tile([1, 512], f32, tag="mp")
                    for t0 in range(0, nk, SB):
                        bn = min(SB, nk - t0)
                        sc = sc_ps.tile([128, SB * 512], f32, tag="sc")
                        spans = []
                        for i in range(bn):
                            tk = t0 + i
                            j = tk - 4 * qb
                            off = 128 * j if j > 0 else 0
                            lo, hi = i * 512 + off, (i + 1) * 512
                            if spans and spans[-1][1] == lo:
                                spans[-1][1] = hi
                            else:
                                spans.append([lo, hi])
                            nc.tensor.matmul(
                                sc[:, lo:hi],
                                kTc[tk // 4][:, (tk % 4) * 128:
                                             (tk % 4 + 1) * 128],
                                qTc[qb][:, off:512],
                                start=True, stop=(j < 0),
                            )
                            if j >= 0:
                                # add -1e30 above the diagonal of the
                                # 128-wide diagonal block
                                nc.tensor.matmul(
                                    sc[:, lo:lo + 128],
                                    ident_bf[:], tri_bf[:],
                                    start=False, stop=True,
                                )
                        probt = ppool.tile([128, SB * 512], mdt, tag="probt")
                        for lo, hi in spans:
                            nc.scalar.activation(
                                probt[:, lo:hi], sc[:, lo:hi],
                                Act.Exp, scale=SCALE,
                            )
                        for i in range(bn):
                            tk = t0 + i
                            j = tk - 4 * qb
                            off = 128 * j if j > 0 else 0
                            nc.tensor.matmul(
                                outT[:, off:512],
                                v_mm[:, tk * 128:(tk + 1) * 128],
                                probt[:, i * 512 + off:(i + 1) * 512],
                                start=(tk == 0), stop=(tk == nk - 1),
                            )
                            nc.tensor.matmul(
                                sums[:, off:512],
                                ones_col[:],
                                probt[:, i * 512 + off:(i + 1) * 512],
                                start=(tk == 0), stop=(tk == nk - 1),
                            )
                    # ---- normalize + store ----
                    sums_sb = spool.tile([1, 512], f32, tag="sums_sb")
                    nc.vector.tensor_copy(sums_sb[:], sums[:])
                    sT = mp_ps.tile([128, 4], f32, tag="mp")
                    for j in range(4):
                        nc.tensor.transpose(
                            sT[:, j:j + 1],
                            sums_sb[0:1, j * 128:(j + 1) * 128],
                            ident[0:1, 0:1],
                        )
                    recip = spool.tile([128, 4], f32, tag="recip")
                    nc.vector.reciprocal(recip[:], sT[:])
                    outT_sb = spool.tile([128, 512], f32, tag="outT_sb")
                    nc.vector.tensor_copy(outT_sb[:], outT[:])
                    o_ps = mp_ps.tile([128, 512], f32, tag="mp")
                    for j in range(4):
                        nc.tensor.transpose(
                            o_ps[:, j * 128:(j + 1) * 128],
                            outT_sb[:, j * 128:(j + 1) * 128],
                            ident[:],
                        )
                    out_sb = spool.tile([128, 512], f32, tag="out_sb")
                    for j in range(4):
                        nc.vector.tensor_scalar_mul(
                            out_sb[:, j * 128:(j + 1) * 128],
                            o_ps[:, j * 128:(j + 1) * 128],
                            recip[:, j:j + 1],
                        )
                    nc.sync.dma_start(
                        od.ap()[bh, qb * 512:(qb + 1) * 512, :]
                        .rearrange("(j p) d -> p j d", p=128),
                        out_sb[:].rearrange("p (j d) -> p j d", d=D),
                    )

    nc.compile()
    return nc


def _get_nc():
    if "nc" not in _CACHE:
        _CACHE["nc"] = _build()
    return _CACHE["nc"]


def _deint_T(x):
    # [N, S, D] -> de-interleave d (evens|odds) then transpose -> [N, D, S]
    return np.ascontiguousarray(
        np.concatenate([x[:, :, 0::2], x[:, :, 1::2]], axis=2)
        .transpose(0, 2, 1))


def _shard(q, k, v, freqs):
    q = np.asarray(q, dtype=np.float32)
    k = np.asarray(k, dtype=np.float32)
    v = np.asarray(v, dtype=np.float32)
    freqs = np.asarray(freqs, dtype=np.float32).reshape(S, D)
    fT = np.ascontiguousarray(
        np.concatenate([freqs[:, 0::2], freqs[:, 1::2]], axis=1).T)
    gT = np.ascontiguousarray(
        np.concatenate([-freqs[:, 1::2], freqs[:, 0::2]], axis=1).T)
    in_maps = []
    for c in range(NCORES):
        h0 = c * HPC

        def bhslice(x):
            # [B, S, Hc, D] -> [B, Hc, S, D] -> [NBH, S, D]
            return np.ascontiguousarray(
                x[:, :, h0:h0 + HPC, :].transpose(0, 2, 1, 3)
            ).reshape(NBH, S, D)

        in_maps.append({
            "qT": _deint_T(bhslice(q)),
            "kT": _deint_T(bhslice(k)),
            "v": bhslice(v),
            "freqsT": fT,
            "freqsG": gT,
        })
    return in_maps


def kernel(q, k, v, freqs):
    nc = _get_nc()
    from concourse.bass_utils import run_bass_kernel_spmd

    in_maps = _shard(q, k, v, freqs)
    res = run_bass_kernel_spmd(nc, in_maps, core_ids=list(range(NCORES)))

    out = np.empty((B, H, S, D), dtype=np.float32)
    for c in range(NCORES):
        h0 = c * HPC
        r = res.results[c]["out"].reshape(B, HPC, S, D)
        out[:, h0:h0 + HPC] = r
    return out
